# revision 52
# baseline (speedup 1.0000x reference)
"""Binarized 3x3 conv + bias + ReLU + eval-mode BatchNorm, Trainium2 Bass kernel.

Problem: x[16,64,256,256] f32, w[64,64,3,3], per-channel b/gamma/beta/mean/var.
  y = BN(relu(conv(sign(x), sign(w)) + b))  (eval-mode BN = per-channel affine)

Strategy (8 NeuronCores, data-parallel over batch, memory-bound target):
  - 2 images per core; image A on SBUF partitions 0-63 (channels), image B on
    64-127.  HBM traffic per core = 32 MiB f32 in + 16 MiB fp16 out (output is
    written as fp16 and upcast to f32 on host; quantization l2-rel ~2e-4).
  - Binarize on-chip as t = (x >= 0) in {1,0} bf16 (one DVE is_ge op); spatial
    padding uses 0.5 so that the identity  conv_pm = 2*conv_t - S  holds exactly
    (S[co] = sum of sign(w) over taps; pads contribute 2*0.5-1 = 0).
  - 3x3 conv = 9 accumulating matmuls per PSUM tile (K=Cin=64, M=Cout=64),
    using 64x64 PE array tiling: 4 quadrants = (imgA,imgB) x (top,bottom
    half-rows) run concurrently -> full 128x128 array utilization.
  - Compute in 8-row units consumed in strict row order (ps_t = unit rows
    0-4, ps_b = rows 4-8, images swapped in ps_b by quadrant geometry) so
    the PE only ever needs a ~10-row input lead; a ~12-matmul dummy warmup
    at t~0 both flips the PE HAM clock-gate (cold 1.2 -> warm 2.4 GHz
    after ~3.4us sustained busy) and delays the first real matmul so the
    input stream builds that lead.
  - Post: ScalarE relu(2*psum + (b-S)) -> f32 SBUF, then VectorE
    y = t*inv + c -> fp16 staging.  The ACT intermediate MUST stay f32: a
    16-bit ACT output dtype was measured to drop the whole core's clock from
    2.4 to 2.0 GHz (MM 376->451ns, LDW 105->126, ACT 1111->1333 — all x1.2).
  - Output: one packed fp16 dram tensor, one 128-partition DMA per block
    (rows natural order; rows with h%8>=4 image-swapped, host unscrambles).
    Output DMAs are deferred ~2 pipeline steps so their DVE-completion
    waits never head-of-line block input chunks; last two blocks' outputs
    drain on SP + SWDGE rings in parallel.
  - Row blocks ramp 8,16,24,32... to track input-DMA arrival and taper
    ...24,16,8 to shorten the drain; blocks DMA exactly their fresh rows in
    8-row chunks, 2 halo rows SBUF-copied from the previous block's tile.
  - Input DMAs ride the Sync HWDGE ring; output DMAs ride the GpSimd SWDGE
    ring.  Measured dead ends (do not revisit): outputs or mid-kernel input
    on the ACT ring (dma_starts cost ~0.65us each on the ACT queue, which
    feeds the PSUM-drain critical path); input split onto the SWDGE ring
    (collapses both rings); fp8 DoubleRow matmuls (walrus ISA check rejects
    tile_position col-group 64 — xdata[3] quadrant HW bug — and 64-wide
    outputs can't fill PSUM's 128 partitions); N=1024 matmuls (PSUM bank
    limit); 48 back-to-back full-array warmup MMs (tripped a whole-kernel
    2.4 -> 2.0 GHz P0 downclock once; possibly environmental).
  - Weights/BN vectors are tiny and prepped on host: lhsT bf16 [128, 9*64]
    (sign(w) transposed to [ci, tap, co], replicated to both partition halves).
  - Measured band on the shared axon TRN2: 169.4-177.2us clean runs
    (baseline was 174-177); environment adds +-4-7us jitter and occasional
    whole-run 2.0 GHz downclocks (+20us) uncorrelated with kernel config.
"""

import numpy as np
import ml_dtypes

import concourse.bass as bass  # noqa: F401  (AP types ride along)
import concourse.mybir as mybir
import concourse.tile as tile
from concourse import bacc
from concourse.bass_utils import run_bass_kernel_spmd

N_CORES = 8
IMGS_PER_CORE = 2
C = 64
H = 256
W = 256
WP = W + 4           # padded row width in xb; data at col offset 2
ROWS_CAP = 34        # xb row capacity (max block 32 + 2 halo)
BN_EPS = 1e-5
DT = mybir.dt

# (r0, nrows) output-row blocks; tapered ends for pipeline fill/drain.
BLOCKS = [(0, 8), (8, 16), (24, 24), (48, 32), (80, 32), (112, 32),
          (144, 32), (176, 32), (208, 24), (232, 16), (248, 8)]
assert sum(r for _, r in BLOCKS) == H

_PROGRAM = None


def _build():
    nc = bacc.Bacc(
        "TRN2",
        target_bir_lowering=False,
        debug=False,
        enable_asserts=False,
    )
    x = nc.dram_tensor("x", [IMGS_PER_CORE, C, H, W], DT.float32, kind="ExternalInput")
    wT = nc.dram_tensor("wT", [128, 9 * 64], DT.bfloat16, kind="ExternalInput")
    bvec = nc.dram_tensor("bvec", [128, 1], DT.float32, kind="ExternalInput")
    ivec = nc.dram_tensor("ivec", [128, 1], DT.float32, kind="ExternalInput")
    cvec = nc.dram_tensor("cvec", [128, 1], DT.float32, kind="ExternalInput")
    # Packed output: one 128-partition tensor; per block, columns hold the
    # top-half rows (partitions = [imgA ch | imgB ch]) followed by the
    # bottom-half rows (partitions = [imgB ch | imgA ch], quadrant-swapped).
    # One large SWDGE DMA per block instead of 1MB + 2x half-partition 0.5MB;
    # the host unscrambles (cheap numpy select).
    ypk = nc.dram_tensor("ypk", [128, H * W], DT.float16, kind="ExternalOutput")

    x_flat = x.ap().rearrange("n c h w -> (n c) (h w)")   # [128, 65536] flat
    ypk_m = ypk.ap()

    with tile.TileContext(nc) as tc:
        with (
            tc.tile_pool(name="consts", bufs=1) as cpool,
            tc.tile_pool(name="xin", bufs=3) as xpool,
            tc.tile_pool(name="xbp", bufs=3) as xbpool,
            tc.tile_pool(name="tsb", bufs=4) as tpool,
            tc.tile_pool(name="yout", bufs=2) as ypool,
            tc.tile_pool(name="psum", bufs=2, space="PSUM") as ppool,
        ):
            wt = cpool.tile([128, 9 * 64], DT.bfloat16, tag="wt")
            bv = cpool.tile([128, 1], DT.float32, tag="bv")
            iv = cpool.tile([128, 1], DT.float32, tag="iv")
            cv = cpool.tile([128, 1], DT.float32, tag="cv")
            wu = cpool.tile([128, 512], DT.bfloat16, tag="wu")

            def load_consts():
                nc.sync.dma_start(wt[:], wT.ap())
                nc.sync.dma_start(bv[:], bvec.ap())
                nc.sync.dma_start(iv[:], ivec.ap())
                nc.sync.dma_start(cv[:], cvec.ap())

            def warm_pe():
                """~12-14us of dummy matmuls at t~0.  Dual purpose: (1) the
                PE HAM clock-gate (cold 1.2 GHz -> warm 2.4 GHz after ~3.4us
                sustained busy) flips before the first real matmul; (2) the
                PE's first real matmul is DELAYED so the input stream builds
                a ~25-row lead -- PE and input run at nearly the same
                per-row rate, so without this head start the PE starves at
                every chunk boundary (and each >3.4us starve re-throttles
                the clock).  PE executes in order, so the warmup chain is
                the delay gate."""
                nc.vector.memset(wu[:], 0.0)
                bridge_pe(12)

            def bridge_pe(n):
                """n dummy matmuls keeping the PE array HAM-busy while early
                ramp blocks' input trickles in; real MMs queue right behind,
                so the cost when data is already present is small, while a
                >3.4us idle window would re-throttle the PE to 1.2 GHz."""
                pw = ppool.tile([128, 1024], DT.float32, tag="pst")
                for i in range(n):
                    nc.tensor.matmul(
                        pw[:, 0:512], wu[:, 0:128], wu[:, 0:512],
                        start=True, stop=True,
                    )

            def fresh_span(bi):
                """x-row span DMAed for block bi (exactly the not-yet-seen rows)."""
                r0, R = BLOCKS[bi]
                f0 = r0 + 1 if r0 > 0 else 0
                f1 = min(r0 + R + 1, H)
                return f0, f1

            def chunks_of(n):
                return [(a, min(a + 8, n)) for a in range(0, n, 8)]

            def dma_block(bi):
                """Issue input DMAs for block bi; allocate its xin/xb tiles."""
                f0, f1 = fresh_span(bi)
                xin = xpool.tile([128, 32 * W], DT.float32, tag="xin")
                xb = xbpool.tile([128, ROWS_CAP * WP], DT.bfloat16, tag="xb")
                # All input on the SP HWDGE ring.  Measured dead ends: input
                # alternated onto the ACT ring raises input bw 296->329 GB/s
                # but each dma_start costs ~0.65us of ACT queue time on the
                # PSUM-drain critical path (-6us net); alternating onto the
                # SWDGE ring collapses both rings (179/216 GB/s, +23us).
                # All input on the SP HWDGE ring.  Measured dead ends:
                # alternating chunks onto the ACT ring raises input bw
                # 296->329 GB/s but costs ~0.65us of ACT queue time per
                # dma_start on the PSUM-drain critical path (net loss);
                # alternating onto the SWDGE ring collapses both rings.
                for a, b in chunks_of(f1 - f0):
                    nc.sync.dma_start(
                        xin[:, a * W : b * W],
                        x_flat[:, (f0 + a) * W : (f0 + b) * W],
                    )
                return xin, xb

            def prep_block(bi, xin, xb, prev_xb_v):
                """Pads, halo copy from previous block, binarize fresh rows.

                xb row k holds binarized x row (r0 - 1 + k), k in [0, R+2).
                """
                r0, R = BLOCKS[bi]
                f0, f1 = fresh_span(bi)
                k0 = 2 if r0 > 0 else 1            # xb row of first fresh x row
                xin_v = xin[:].rearrange("p (r c) -> p r c", c=W)
                xb_v = xb[:].rearrange("p (r c) -> p r c", c=WP)
                nc.vector.memset(xb_v[:, :, 0:2], 0.5)
                nc.vector.memset(xb_v[:, :, 2 + W : WP], 0.5)
                if r0 == 0:
                    nc.vector.memset(xb_v[:, 0:1, :], 0.5)
                else:
                    prevR = BLOCKS[bi - 1][1]
                    nc.vector.tensor_copy(
                        xb_v[:, 0:2, :], prev_xb_v[:, prevR : prevR + 2, :]
                    )
                if r0 + R == H:
                    nc.vector.memset(xb_v[:, R + 1 : R + 2, :], 0.5)
                for a, b in chunks_of(f1 - f0):
                    nc.vector.tensor_scalar(
                        xb_v[:, k0 + a : k0 + b, 2 : 2 + W],
                        xin_v[:, a:b, :],
                        0.0,
                        None,
                        op0=mybir.AluOpType.is_ge,
                    )
                return xb_v

            def compute_block(bi, xb_v):
                """Matmuls + post-ops + output DMAs for a prepared block.

                Units of 4 output rows, consumed in strict row order so the
                PE only ever needs a ~6-row input lead (no half-block
                front-loading).  Quadrants split the ROW along columns:
                ps_t = [A-left | B-left], ps_b = [B-right | A-right]
                (right halves image-swapped by quadrant geometry; host
                unscrambles).  Each psum tile is 1 bank; bufs=4 -> all 8
                banks in flight.  Staging per unit: [left 512 | right 512].
                """
                r0, R = BLOCKS[bi]
                y_ = ypool.tile([128, 32 * W], DT.float16, tag="yst")
                for u in range(R // 8):              # 8-row units, row-ordered
                    ps_t = ppool.tile([128, 1024], DT.float32, tag="pst")
                    ps_b = ppool.tile([128, 1024], DT.float32, tag="psb")
                    for sub in range(2):             # 2 rows per matmul set
                        c0 = sub * 512
                        for t in range(9):
                            dy, dx = divmod(t, 3)
                            first, last = (t == 0), (t == 8)
                            rt = 8 * u + 2 * sub + dy       # rows 8u..8u+4
                            rb_ = 8 * u + 4 + 2 * sub + dy  # rows 8u+4..8u+8
                            cs = 1 + dx
                            quads = (
                                (ps_t, 0, 0, rt),      # A-rows04 -> psT[0:64]
                                (ps_t, 64, 64, rt),    # B-rows04 -> psT[64:128]
                                (ps_b, 64, 0, rb_),    # B-rows48 -> psB[0:64]
                                (ps_b, 0, 64, rb_),    # A-rows48 -> psB[64:128]
                            )
                            for ps, xp0, op0_, rlo in quads:
                                wslc = wt[xp0 : xp0 + 64, t * 64 : (t + 1) * 64]
                                rhs = xb_v[xp0 : xp0 + 64, rlo : rlo + 2, cs : cs + W]
                                nc.tensor.matmul(
                                    ps[op0_ : op0_ + 64, c0 : c0 + 512],
                                    wslc,
                                    rhs,
                                    start=first,
                                    stop=last,
                                )
                    for ps, off in ((ps_t, 8 * u * W), (ps_b, (8 * u + 4) * W)):
                        tsb = tpool.tile([128, 1024], DT.float32, tag="tsb")
                        nc.scalar.activation(
                            tsb[:],
                            ps[:],
                            mybir.ActivationFunctionType.Relu,
                            bias=bv[:],
                            scale=2.0,
                        )
                        nc.vector.tensor_scalar(
                            y_[:, off : off + 1024],
                            tsb[:],
                            iv[:],
                            cv[:],
                            op0=mybir.AluOpType.mult,
                            op1=mybir.AluOpType.add,
                        )
                return (bi, r0, R, y_)

            # Software pipeline: only the input DMAs of block b+1 are issued
            # ahead of block b's compute; binarize of b+1 is emitted after
            # block b's PSUM-drain posts so the DVE FIFO never stalls a drain
            # behind a DMA wait.
            def flush_out(ent):
                """Output DMA, deferred ~2 iterations after its compute was
                emitted so its DVE-completion wait never head-of-line
                blocks later input chunks on the shared SWDGE ring.  The
                last two blocks ride the (by then idle) SP ring so the
                final drain uses two rings in parallel."""
                obi, r0, R, y_ = ent
                # The last three blocks' outputs ride the SP HWDGE ring
                # (lower latency than SWDGE, and input is done by then):
                # they are emitted after the final input chunks, so their
                # DVE waits cannot head-of-line block any input DMA.
                oeng = nc.sync if obi >= len(BLOCKS) - 3 else nc.gpsimd
                oeng.dma_start(
                    ypk_m[:, r0 * W : (r0 + R) * W], y_[:, 0 : R * W]
                )

            pending = None
            prev_xb_v = None
            outq = []
            for bi in range(len(BLOCKS)):
                xin, xb = dma_block(bi)
                if bi == 0:
                    load_consts()
                    warm_pe()
                if outq:
                    flush_out(outq.pop(0))
                if pending is not None:
                    outq.append(compute_block(pending[0], pending[1]))
                xb_v = prep_block(bi, xin, xb, prev_xb_v)
                prev_xb_v = xb_v
                pending = (bi, xb_v)
            outq.append(compute_block(pending[0], pending[1]))
            for ent in outq:
                flush_out(ent)
    nc.compile()
    return nc


def _get_program():
    global _PROGRAM
    if _PROGRAM is None:
        _PROGRAM = _build()
    return _PROGRAM


def _prep_params(w, b, gamma, beta, running_mean, running_var):
    wb = np.where(w >= 0, 1.0, -1.0).astype(np.float32)          # [co, ci, ky, kx]
    wt = np.ascontiguousarray(wb.transpose(1, 2, 3, 0))          # [ci, ky, kx, co]
    wt = wt.reshape(C, 9 * C).astype(ml_dtypes.bfloat16)
    wt2 = np.ascontiguousarray(np.concatenate([wt, wt], axis=0))  # [128, 576]
    s = wb.sum(axis=(1, 2, 3)).astype(np.float32)
    inv = (gamma.astype(np.float32) / np.sqrt(running_var.astype(np.float32) + BN_EPS)).astype(np.float32)
    cc = (beta.astype(np.float32) - running_mean.astype(np.float32) * inv).astype(np.float32)
    bp = (b.astype(np.float32) - s).astype(np.float32)

    def rep(v):
        return np.ascontiguousarray(np.tile(v.astype(np.float32), 2).reshape(128, 1))

    return wt2, rep(bp), rep(inv), rep(cc)


def _unpack_y(ypk):
    """[128, H*W] packed fp16 -> [2, C, H, W] f32.

    Rows are natural order; rows with (h % 8) >= 4 came from the ps_b
    quadrants and have the two images' partitions swapped.
    """
    p = np.asarray(ypk).reshape(128, H, W).astype(np.float32)
    a, bb = p[:64], p[64:]
    sw = (np.arange(H) % 8 >= 4)[None, :, None]
    return np.stack([np.where(sw, bb, a), np.where(sw, a, bb)])


def run(x, w, b, gamma, beta, running_mean, running_var, trace=False):
    nc = _get_program()
    wt2, bp, inv, cc = _prep_params(w, b, gamma, beta, running_mean, running_var)
    x = np.asarray(x, dtype=np.float32)
    in_maps = []
    for i in range(N_CORES):
        in_maps.append(
            {
                "x": np.ascontiguousarray(x[IMGS_PER_CORE * i : IMGS_PER_CORE * (i + 1)]),
                "wT": wt2,
                "bvec": bp,
                "ivec": inv,
                "cvec": cc,
            }
        )
    res = run_bass_kernel_spmd(nc, in_maps, list(range(N_CORES)), trace=trace)
    y = np.concatenate(
        [_unpack_y(res.results[i]["ypk"]) for i in range(N_CORES)],
        axis=0,
    )
    return y, res


def kernel(x, w, b, gamma, beta, running_mean, running_var):
    y, _ = run(x, w, b, gamma, beta, running_mean, running_var)
    return y



# revision 53
# speedup vs baseline: 1.0964x; 1.0964x over previous
"""Binarized 3x3 conv + bias + ReLU + eval-mode BatchNorm, Trainium2 Bass kernel.

Problem: x[16,64,256,256] f32, w[64,64,3,3], per-channel b/gamma/beta/mean/var.
  y = BN(relu(conv(sign(x), sign(w)) + b))  (eval-mode BN = per-channel affine)

Strategy (8 NeuronCores, data-parallel over batch, memory-bound target):
  - 2 images per core; image A on SBUF partitions 0-63 (channels), image B on
    64-127.  HBM traffic per core = 32 MiB f32 in + 16 MiB fp16 out (output is
    written as fp16 and upcast to f32 on host; quantization l2-rel ~2e-4).
  - Binarize on-chip as t = (x >= 0) in {1,0} bf16 (one DVE is_ge op); spatial
    padding uses 0.5 so that the identity  conv_pm = 2*conv_t - S  holds exactly
    (S[co] = sum of sign(w) over taps; pads contribute 2*0.5-1 = 0).
  - 3x3 conv = 9 accumulating matmuls per PSUM tile (K=Cin=64, M=Cout=64),
    using 64x64 PE array tiling: 4 quadrants = (imgA,imgB) x (top,bottom
    half-rows) run concurrently -> full 128x128 array utilization.
  - Compute in 8-row units consumed in strict row order (ps_t = unit rows
    0-4, ps_b = rows 4-8, images swapped in ps_b by quadrant geometry) so
    the PE only ever needs a ~10-row input lead; a ~12-matmul dummy warmup
    at t~0 both flips the PE HAM clock-gate (cold 1.2 -> warm 2.4 GHz
    after ~3.4us sustained busy) and delays the first real matmul so the
    input stream builds that lead.
  - Post: ScalarE relu(2*psum + (b-S)) -> f32 SBUF, then VectorE
    y = t*inv + c -> fp16 staging.  The ACT intermediate MUST stay f32: a
    16-bit ACT output dtype was measured to drop the whole core's clock from
    2.4 to 2.0 GHz (MM 376->451ns, LDW 105->126, ACT 1111->1333 — all x1.2).
  - Output: one packed fp16 dram tensor, one 128-partition DMA per block
    (rows natural order; rows with h%8>=4 image-swapped, host unscrambles).
    Output DMAs are deferred ~2 pipeline steps so their DVE-completion
    waits never head-of-line block input chunks; last two blocks' outputs
    drain on SP + SWDGE rings in parallel.
  - Row blocks ramp 8,16,24,32... to track input-DMA arrival and taper
    ...24,16,8 to shorten the drain; blocks DMA exactly their fresh rows in
    8-row chunks, 2 halo rows SBUF-copied from the previous block's tile.
  - Input DMAs ride the Sync HWDGE ring; output DMAs ride the GpSimd SWDGE
    ring.  Measured dead ends (do not revisit): outputs or mid-kernel input
    on the ACT ring (dma_starts cost ~0.65us each on the ACT queue, which
    feeds the PSUM-drain critical path); input split onto the SWDGE ring
    (collapses both rings); fp8 DoubleRow matmuls (walrus ISA check rejects
    tile_position col-group 64 — xdata[3] quadrant HW bug — and 64-wide
    outputs can't fill PSUM's 128 partitions); N=1024 matmuls (PSUM bank
    limit); 48 back-to-back full-array warmup MMs (tripped a whole-kernel
    2.4 -> 2.0 GHz P0 downclock once; possibly environmental).
  - Weights/BN vectors are tiny and prepped on host: lhsT bf16 [128, 9*64]
    (sign(w) transposed to [ci, tap, co], replicated to both partition halves).
  - Measured band on the shared axon TRN2: 169.4-177.2us clean runs
    (baseline was 174-177); environment adds +-4-7us jitter and occasional
    whole-run 2.0 GHz downclocks (+20us) uncorrelated with kernel config.
"""

import numpy as np
import ml_dtypes

import concourse.bass as bass  # noqa: F401  (AP types ride along)
import concourse.mybir as mybir
import concourse.tile as tile
from concourse import bacc
from concourse.bass_utils import run_bass_kernel_spmd

N_CORES = 8
IMGS_PER_CORE = 2
C = 64
H = 256
W = 256
WP = W + 4           # padded row width in xb; data at col offset 2
ROWS_CAP = 34        # xb row capacity (max block 32 + 2 halo)
BN_EPS = 1e-5
DT = mybir.dt

# (r0, nrows) output-row blocks; tapered ends for pipeline fill/drain.
BLOCKS = [(0, 8), (8, 16), (24, 24), (48, 32), (80, 32), (112, 32),
          (144, 32), (176, 32), (208, 24), (232, 16), (248, 8)]
assert sum(r for _, r in BLOCKS) == H

_PROGRAM = None


def _build():
    nc = bacc.Bacc(
        "TRN2",
        target_bir_lowering=False,
        debug=False,
        enable_asserts=False,
    )
    x = nc.dram_tensor("x", [IMGS_PER_CORE, C, H, W], DT.float32, kind="ExternalInput")
    wT = nc.dram_tensor("wT", [128, 9 * 64], DT.bfloat16, kind="ExternalInput")
    bvec = nc.dram_tensor("bvec", [128, 1], DT.float32, kind="ExternalInput")
    ivec = nc.dram_tensor("ivec", [128, 1], DT.float32, kind="ExternalInput")
    cvec = nc.dram_tensor("cvec", [128, 1], DT.float32, kind="ExternalInput")
    # Packed output: one 128-partition tensor; per block, columns hold the
    # top-half rows (partitions = [imgA ch | imgB ch]) followed by the
    # bottom-half rows (partitions = [imgB ch | imgA ch], quadrant-swapped).
    # One large SWDGE DMA per block instead of 1MB + 2x half-partition 0.5MB;
    # the host unscrambles (cheap numpy select).
    ypk = nc.dram_tensor("ypk", [128, H * W], DT.float16, kind="ExternalOutput")

    x_flat = x.ap().rearrange("n c h w -> (n c) (h w)")   # [128, 65536] flat
    ypk_m = ypk.ap()

    with tile.TileContext(nc) as tc:
        with (
            tc.tile_pool(name="consts", bufs=1) as cpool,
            tc.tile_pool(name="xin", bufs=3) as xpool,
            tc.tile_pool(name="xbp", bufs=3) as xbpool,
            tc.tile_pool(name="tsb", bufs=4) as tpool,
            tc.tile_pool(name="yout", bufs=2) as ypool,
            tc.tile_pool(name="psum", bufs=2, space="PSUM") as ppool,
        ):
            wt = cpool.tile([128, 9 * 64], DT.bfloat16, tag="wt")
            bv = cpool.tile([128, 1], DT.float32, tag="bv")
            iv = cpool.tile([128, 1], DT.float32, tag="iv")
            cv = cpool.tile([128, 1], DT.float32, tag="cv")
            wu = cpool.tile([128, 512], DT.bfloat16, tag="wu")

            def load_consts():
                nc.sync.dma_start(wt[:], wT.ap())
                nc.sync.dma_start(bv[:], bvec.ap())
                nc.sync.dma_start(iv[:], ivec.ap())
                nc.sync.dma_start(cv[:], cvec.ap())

            def warm_pe():
                """~12-14us of dummy matmuls at t~0.  Dual purpose: (1) the
                PE HAM clock-gate (cold 1.2 GHz -> warm 2.4 GHz after ~3.4us
                sustained busy) flips before the first real matmul; (2) the
                PE's first real matmul is DELAYED so the input stream builds
                a ~25-row lead -- PE and input run at nearly the same
                per-row rate, so without this head start the PE starves at
                every chunk boundary (and each >3.4us starve re-throttles
                the clock).  PE executes in order, so the warmup chain is
                the delay gate."""
                nc.vector.memset(wu[:], 0.0)
                bridge_pe(12)

            def bridge_pe(n):
                """n dummy matmuls keeping the PE array HAM-busy while early
                ramp blocks' input trickles in; real MMs queue right behind,
                so the cost when data is already present is small, while a
                >3.4us idle window would re-throttle the PE to 1.2 GHz."""
                pw = ppool.tile([128, 1024], DT.float32, tag="pst")
                for i in range(n):
                    nc.tensor.matmul(
                        pw[:, 0:512], wu[:, 0:128], wu[:, 0:512],
                        start=True, stop=True,
                    )

            def fresh_span(bi):
                """x-row span DMAed for block bi (exactly the not-yet-seen rows)."""
                r0, R = BLOCKS[bi]
                f0 = r0 + 1 if r0 > 0 else 0
                f1 = min(r0 + R + 1, H)
                return f0, f1

            def chunks_of(n):
                return [(a, min(a + 8, n)) for a in range(0, n, 8)]

            def dma_block(bi):
                """Issue input DMAs for block bi; allocate its xin/xb tiles."""
                f0, f1 = fresh_span(bi)
                xin = xpool.tile([128, 32 * W], DT.float32, tag="xin")
                xb = xbpool.tile([128, ROWS_CAP * WP], DT.bfloat16, tag="xb")
                # All input on the SP HWDGE ring.  Measured dead ends: input
                # alternated onto the ACT ring raises input bw 296->329 GB/s
                # but each dma_start costs ~0.65us of ACT queue time on the
                # PSUM-drain critical path (-6us net); alternating onto the
                # SWDGE ring collapses both rings (179/216 GB/s, +23us).
                # All input on the SP HWDGE ring.  Measured dead ends:
                # alternating chunks onto the ACT ring raises input bw
                # 296->329 GB/s but costs ~0.65us of ACT queue time per
                # dma_start on the PSUM-drain critical path (net loss);
                # alternating onto the SWDGE ring collapses both rings.
                for a, b in chunks_of(f1 - f0):
                    nc.sync.dma_start(
                        xin[:, a * W : b * W],
                        x_flat[:, (f0 + a) * W : (f0 + b) * W],
                    )
                return xin, xb

            def prep_block(bi, xin, xb, prev_xb_v):
                """Pads, halo copy from previous block, binarize fresh rows.

                xb row k holds binarized x row (r0 - 1 + k), k in [0, R+2).
                """
                r0, R = BLOCKS[bi]
                f0, f1 = fresh_span(bi)
                k0 = 2 if r0 > 0 else 1            # xb row of first fresh x row
                xin_v = xin[:].rearrange("p (r c) -> p r c", c=W)
                xb_v = xb[:].rearrange("p (r c) -> p r c", c=WP)
                nc.vector.memset(xb_v[:, :, 0:2], 0.5)
                nc.vector.memset(xb_v[:, :, 2 + W : WP], 0.5)
                if r0 == 0:
                    nc.vector.memset(xb_v[:, 0:1, :], 0.5)
                else:
                    prevR = BLOCKS[bi - 1][1]
                    nc.vector.tensor_copy(
                        xb_v[:, 0:2, :], prev_xb_v[:, prevR : prevR + 2, :]
                    )
                if r0 + R == H:
                    nc.vector.memset(xb_v[:, R + 1 : R + 2, :], 0.5)
                for a, b in chunks_of(f1 - f0):
                    nc.vector.tensor_scalar(
                        xb_v[:, k0 + a : k0 + b, 2 : 2 + W],
                        xin_v[:, a:b, :],
                        0.0,
                        None,
                        op0=mybir.AluOpType.is_ge,
                    )
                return xb_v

            def compute_block(bi, xb_v):
                """Matmuls + post-ops + output DMAs for a prepared block.

                Units of 4 output rows, consumed in strict row order so the
                PE only ever needs a ~6-row input lead (no half-block
                front-loading).  Quadrants split the ROW along columns:
                ps_t = [A-left | B-left], ps_b = [B-right | A-right]
                (right halves image-swapped by quadrant geometry; host
                unscrambles).  Each psum tile is 1 bank; bufs=4 -> all 8
                banks in flight.  Staging per unit: [left 512 | right 512].
                """
                r0, R = BLOCKS[bi]
                y_ = ypool.tile([128, 32 * W], DT.float16, tag="yst")
                for u in range(R // 8):              # 8-row units, row-ordered
                    ps_t = ppool.tile([128, 1024], DT.float32, tag="pst")
                    ps_b = ppool.tile([128, 1024], DT.float32, tag="psb")
                    for sub in range(2):             # 2 rows per matmul set
                        c0 = sub * 512
                        for t in range(9):
                            dy, dx = divmod(t, 3)
                            first, last = (t == 0), (t == 8)
                            rt = 8 * u + 2 * sub + dy       # rows 8u..8u+4
                            rb_ = 8 * u + 4 + 2 * sub + dy  # rows 8u+4..8u+8
                            cs = 1 + dx
                            quads = (
                                (ps_t, 0, 0, rt),      # A-rows04 -> psT[0:64]
                                (ps_t, 64, 64, rt),    # B-rows04 -> psT[64:128]
                                (ps_b, 64, 0, rb_),    # B-rows48 -> psB[0:64]
                                (ps_b, 0, 64, rb_),    # A-rows48 -> psB[64:128]
                            )
                            for ps, xp0, op0_, rlo in quads:
                                wslc = wt[xp0 : xp0 + 64, t * 64 : (t + 1) * 64]
                                rhs = xb_v[xp0 : xp0 + 64, rlo : rlo + 2, cs : cs + W]
                                nc.tensor.matmul(
                                    ps[op0_ : op0_ + 64, c0 : c0 + 512],
                                    wslc,
                                    rhs,
                                    start=first,
                                    stop=last,
                                )
                    for ps, off in ((ps_t, 8 * u * W), (ps_b, (8 * u + 4) * W)):
                        tsb = tpool.tile([128, 1024], DT.float32, tag="tsb")
                        nc.scalar.activation(
                            tsb[:],
                            ps[:],
                            mybir.ActivationFunctionType.Relu,
                            bias=bv[:],
                            scale=2.0,
                        )
                        nc.vector.tensor_scalar(
                            y_[:, off : off + 1024],
                            tsb[:],
                            iv[:],
                            cv[:],
                            op0=mybir.AluOpType.mult,
                            op1=mybir.AluOpType.add,
                        )
                return (bi, r0, R, y_)

            # Software pipeline: only the input DMAs of block b+1 are issued
            # ahead of block b's compute; binarize of b+1 is emitted after
            # block b's PSUM-drain posts so the DVE FIFO never stalls a drain
            # behind a DMA wait.
            def flush_out(ent):
                """Output DMA, deferred ~2 iterations after its compute was
                emitted so its DVE-completion wait never head-of-line
                blocks later input chunks on the shared SWDGE ring.  The
                last two blocks ride the (by then idle) SP ring so the
                final drain uses two rings in parallel."""
                obi, r0, R, y_ = ent
                # The last two blocks' outputs ride the SP HWDGE ring
                # (lower latency than SWDGE, and input is done by then):
                # they are emitted after the final input chunks, so their
                # DVE waits cannot head-of-line block any input DMA.
                oeng = nc.sync if obi >= len(BLOCKS) - 2 else nc.gpsimd
                oeng.dma_start(
                    ypk_m[:, r0 * W : (r0 + R) * W], y_[:, 0 : R * W]
                )

            pending = None
            prev_xb_v = None
            outq = []
            for bi in range(len(BLOCKS)):
                xin, xb = dma_block(bi)
                if bi == 0:
                    load_consts()
                    warm_pe()
                if outq:
                    flush_out(outq.pop(0))
                if pending is not None:
                    outq.append(compute_block(pending[0], pending[1]))
                xb_v = prep_block(bi, xin, xb, prev_xb_v)
                prev_xb_v = xb_v
                pending = (bi, xb_v)
            outq.append(compute_block(pending[0], pending[1]))
            for ent in outq:
                flush_out(ent)
    nc.compile()
    return nc


def _get_program():
    global _PROGRAM
    if _PROGRAM is None:
        _PROGRAM = _build()
    return _PROGRAM


def _prep_params(w, b, gamma, beta, running_mean, running_var):
    wb = np.where(w >= 0, 1.0, -1.0).astype(np.float32)          # [co, ci, ky, kx]
    wt = np.ascontiguousarray(wb.transpose(1, 2, 3, 0))          # [ci, ky, kx, co]
    wt = wt.reshape(C, 9 * C).astype(ml_dtypes.bfloat16)
    wt2 = np.ascontiguousarray(np.concatenate([wt, wt], axis=0))  # [128, 576]
    s = wb.sum(axis=(1, 2, 3)).astype(np.float32)
    inv = (gamma.astype(np.float32) / np.sqrt(running_var.astype(np.float32) + BN_EPS)).astype(np.float32)
    cc = (beta.astype(np.float32) - running_mean.astype(np.float32) * inv).astype(np.float32)
    bp = (b.astype(np.float32) - s).astype(np.float32)

    def rep(v):
        return np.ascontiguousarray(np.tile(v.astype(np.float32), 2).reshape(128, 1))

    return wt2, rep(bp), rep(inv), rep(cc)


def _unpack_y(ypk):
    """[128, H*W] packed fp16 -> [2, C, H, W] f32.

    Rows are natural order; rows with (h % 8) >= 4 came from the ps_b
    quadrants and have the two images' partitions swapped.
    """
    p = np.asarray(ypk).reshape(128, H, W).astype(np.float32)
    a, bb = p[:64], p[64:]
    sw = (np.arange(H) % 8 >= 4)[None, :, None]
    return np.stack([np.where(sw, bb, a), np.where(sw, a, bb)])


def run(x, w, b, gamma, beta, running_mean, running_var, trace=False):
    nc = _get_program()
    wt2, bp, inv, cc = _prep_params(w, b, gamma, beta, running_mean, running_var)
    x = np.asarray(x, dtype=np.float32)
    in_maps = []
    for i in range(N_CORES):
        in_maps.append(
            {
                "x": np.ascontiguousarray(x[IMGS_PER_CORE * i : IMGS_PER_CORE * (i + 1)]),
                "wT": wt2,
                "bvec": bp,
                "ivec": inv,
                "cvec": cc,
            }
        )
    res = run_bass_kernel_spmd(nc, in_maps, list(range(N_CORES)), trace=trace)
    y = np.concatenate(
        [_unpack_y(res.results[i]["ypk"]) for i in range(N_CORES)],
        axis=0,
    )
    return y, res


def kernel(x, w, b, gamma, beta, running_mean, running_var):
    y, _ = run(x, w, b, gamma, beta, running_mean, running_var)
    return y

